# revision 25
# baseline (speedup 1.0000x reference)
"""Causal self-attention (B=1, T=4096, C=768, H=12) on 8 Trainium2 NeuronCores.

Sharding (uniform SPMD program, zero dummy work):
  - heads 0-7 live whole on cores 0-7 ("slot A", full softmax on device)
  - heads 8-11 are split between core pairs (c, c+4) by k-block PARITY:
    core c<4 handles even 128-row k-blocks, core c+4 the odd ones, each
    producing an UNNORMALIZED partial y plus the partial softmax
    denominator ("slot B"). The host adds the two partials and divides --
    exact because no max-subtraction is used (scores are small).
    The parity selection is pure data: the host packs x^T's k-blocks of
    the matching parity contiguously (xp), so both program variants are
    the same instruction stream.

Per core the device program:
  1. loads host-pretransposed x^T (bf16) -- no on-chip transposes of x
  2. projects Q^T/K^T (heads on partitions) and V in natural [t, d]
     layout (direct, no V transpose)
  3. causal attention per 256-row q-block: S^T = K^T.T Q^T per 128-wide
     k-block, exp on the scalar engine in 1536-wide batches, diagonal
     masks on gpsimd, then P^T.T V accumulated as y[q, d] with an extra
     ones-column in V giving the softmax denominator in column 64
  4. slot A: normalize y, transpose 128x64 tiles via the PE array, and
     project with this head's 64 rows of w_proj; slot B: ship raw y+denom
  The host sums the 8 partial projections and adds heads 8-11's
  contribution (a small [4096,256]x[256,768] matmul) in fp32.

All matmul inputs are bf16 (fp32 PSUM accumulation); the relative error
vs the fp32 reference stays ~1e-3, well inside the 2e-2 gate.
"""

import sys

sys.path.insert(0, "/opt/trn_rl_repo")

import numpy as np

T = 4096
C = 768
H = 12
HD = 64
N_CORES = 8
TS = 512  # t-slice for x load / projection
NTS = T // TS  # 8
QB = 256  # q-block rows
NQB = T // QB  # 16
KB = 128  # k-block rows
NKB = T // KB  # 32
GRP = 4  # k-blocks per score/exp group (4*256 = 1024 wide = 2 PSUM banks)

_CACHE = {}


def _groups(n):
    """Chunk k-block indices 0..n-1 into groups of GRP (ascending)."""
    return [list(range(g, min(g + GRP, n))) for g in range(0, n, GRP)]


def _build_nc():
    import concourse.bacc as bacc
    import concourse.tile as tile
    import concourse.mybir as mybir
    from concourse.masks import make_identity
    from contextlib import ExitStack
    import collections

    F32 = mybir.dt.float32
    BF16 = mybir.dt.bfloat16
    EXP = mybir.ActivationFunctionType.Exp
    GE = mybir.AluOpType.is_ge

    nc = bacc.Bacc(
        "TRN2",
        target_bir_lowering=False,
        debug=False,
        enable_asserts=True,
        num_devices=N_CORES,
    )
    # host-pretransposed x^T, full [C, T]
    xt_d = nc.dram_tensor("xt", [C, T], BF16, kind="ExternalInput")
    # x^T with only this core's parity of k-blocks, packed [C, T//2]
    xp_d = nc.dram_tensor("xp", [C, T // 2], BF16, kind="ExternalInput")
    wq_d = nc.dram_tensor("wq", [C, 2 * HD], BF16, kind="ExternalInput")
    wk_d = nc.dram_tensor("wk", [C, 2 * HD], BF16, kind="ExternalInput")
    wv_d = nc.dram_tensor("wv", [C, 2 * HD], BF16, kind="ExternalInput")
    wp_d = nc.dram_tensor("wp", [HD, C], BF16, kind="ExternalInput")
    mb_d = nc.dram_tensor("maskb", [KB, QB], BF16, kind="ExternalInput")
    out_d = nc.dram_tensor("out", [T, C], BF16, kind="ExternalOutput")
    yb_d = nc.dram_tensor("yb", [T, HD + 1], BF16, kind="ExternalOutput")

    scale = 1.0 / float(np.sqrt(HD))

    with ExitStack() as ctx:
        tc = ctx.enter_context(tile.TileContext(nc))
        singles = ctx.enter_context(tc.tile_pool(name="singles", bufs=1))
        ptpool = ctx.enter_context(tc.tile_pool(name="ptpool", bufs=8))
        opool = ctx.enter_context(tc.tile_pool(name="opool", bufs=2))
        rpool = ctx.enter_context(tc.tile_pool(name="rpool", bufs=4))
        ps_big = ctx.enter_context(tc.tile_pool(name="ps_big", bufs=3, space="PSUM"))
        ps_y = ctx.enter_context(tc.tile_pool(name="ps_y", bufs=2, space="PSUM"))

        # ---- persistent SBUF tensors ----
        xt = [singles.tile([128, T], BF16, name=f"xt{c}") for c in range(6)]
        xp = [singles.tile([128, T // 2], BF16, name=f"xp{c}") for c in range(6)]
        qt = singles.tile([128, T], BF16)  # rows 0:64 head A, 64:128 head B
        kt = singles.tile([128, T], BF16)  # B rows use cols 0:T//2 (packed)
        vA = singles.tile([128, NKB, HD + 1], BF16)
        y_sb = singles.tile([128, (T // KB) * HD], BF16)  # normalized y, head A
        vB = singles.tile([128, NKB // 2, HD + 1], BF16)
        wq_sb = singles.tile([128, 6, 2 * HD], BF16)
        wk_sb = singles.tile([128, 6, 2 * HD], BF16)
        wv_sb = singles.tile([128, 6, 2 * HD], BF16)
        wp_sb = singles.tile([HD, C], BF16)
        maskb = singles.tile([KB, QB], BF16)
        ident = singles.tile([128, 128], BF16)

        ident_f32 = singles.tile([128, 128], F32)
        make_identity(nc, ident_f32)
        nc.vector.tensor_copy(out=ident, in_=ident_f32)
        nc.gpsimd.memset(vA[:, :, HD : HD + 1], 1.0)
        nc.gpsimd.memset(vB[:, :, HD : HD + 1], 1.0)

        # ---- weight + mask DMAs (vector-engine queue; tiny) ----
        nc.scalar.dma_start(
            out=wq_sb, in_=wq_d.ap().rearrange("(c p) j -> p c j", p=128)
        )
        nc.scalar.dma_start(
            out=wk_sb, in_=wk_d.ap().rearrange("(c p) j -> p c j", p=128)
        )
        nc.scalar.dma_start(
            out=wv_sb, in_=wv_d.ap().rearrange("(c p) j -> p c j", p=128)
        )
        nc.scalar.dma_start(out=wp_sb, in_=wp_d.ap())
        nc.scalar.dma_start(out=maskb, in_=mb_d.ap())

        # ---- x^T input DMAs (sync-engine queue), finest slices first ----
        for c in range(6):
            nc.sync.dma_start(
                out=xt[c][:, 0:TS], in_=xt_d.ap()[128 * c : 128 * (c + 1), 0:TS]
            )
        for c in range(6):
            nc.sync.dma_start(
                out=xp[c][:, 0:1024], in_=xp_d.ap()[128 * c : 128 * (c + 1), 0:1024]
            )
        for c in range(6):
            nc.sync.dma_start(
                out=xt[c][:, TS : 2 * TS],
                in_=xt_d.ap()[128 * c : 128 * (c + 1), TS : 2 * TS],
            )
        for c in range(6):
            nc.sync.dma_start(
                out=xt[c][:, 1024:2048],
                in_=xt_d.ap()[128 * c : 128 * (c + 1), 1024:2048],
            )
        for c in range(6):
            nc.sync.dma_start(
                out=xt[c][:, 2048:3072],
                in_=xt_d.ap()[128 * c : 128 * (c + 1), 2048:3072],
            )
        for c in range(6):
            nc.sync.dma_start(
                out=xp[c][:, 1024:2048],
                in_=xp_d.ap()[128 * c : 128 * (c + 1), 1024:2048],
            )
        for c in range(6):
            nc.sync.dma_start(
                out=xt[c][:, 3072:4096],
                in_=xt_d.ap()[128 * c : 128 * (c + 1), 3072:4096],
            )

        # ---- emission helpers ----
        work_q = collections.deque()  # deferred closures to fill PE gaps

        # projection emitted as small "pieces" (<=6 matmuls) so interleaved
        # attention score groups are never stuck behind a long proj burst in
        # PE program order. Pieces of one logical chain share a PSUM tile via
        # the shared dict; skip_group_check allows interleaved accumulation.
        def mk_main_pieces(s):
            c0, c1 = s * TS, (s + 1) * TS
            state = {}

            def qk_piece(cs, ce, copy):
                def go():
                    if "big" not in state:
                        state["big"] = ps_big.tile(
                            [128, 1024], F32, name="big", tag="big"
                        )
                    big = state["big"]
                    for c in range(cs, ce):
                        nc.tensor.matmul(
                            big[:, 0:512],
                            lhsT=wq_sb[:, c, :],
                            rhs=xt[c][:, c0:c1],
                            start=(c == 0),
                            stop=(c == 5),
                            skip_group_check=True,
                        )
                        nc.tensor.matmul(
                            big[0:64, 512:1024],
                            lhsT=wk_sb[:, c, 0:HD],
                            rhs=xt[c][:, c0:c1],
                            start=(c == 0),
                            stop=(c == 5),
                            skip_group_check=True,
                        )
                    if copy:
                        nc.vector.tensor_copy(out=qt[:, c0:c1], in_=big[:, 0:512])
                        nc.vector.tensor_copy(
                            out=kt[0:64, c0:c1], in_=big[0:64, 512:1024]
                        )

                return go

            def v_piece(j0, j1, copy):
                def go():
                    if "bigv" not in state:
                        state["bigv"] = ps_big.tile(
                            [128, 1024], F32, name="bigv", tag="big"
                        )
                    bigv = state["bigv"]
                    for j in range(j0, j1):
                        t0 = c0 + 128 * j
                        for c in range(6):
                            nc.tensor.matmul(
                                bigv[:, 64 * j : 64 * (j + 1)],
                                lhsT=xt[c][:, t0 : t0 + 128],
                                rhs=wv_sb[:, c, 0:HD],
                                start=(c == 0),
                                stop=(c == 5),
                                skip_group_check=True,
                            )
                    if copy:
                        nc.vector.tensor_copy(
                            out=vA[:, 4 * s : 4 * s + 4, 0:HD],
                            in_=bigv[:, 0:256].rearrange("p (j d) -> p j d", j=4),
                        )

                return go

            return [
                qk_piece(0, 3, False),
                qk_piece(3, 6, True),
                v_piece(0, 2, False),
                v_piece(2, 4, True),
            ]

        def mk_packed_pieces(p):
            c0, c1 = p * TS, (p + 1) * TS
            state = {}

            def k_piece(cs, ce, copy):
                def go():
                    if "big" not in state:
                        state["big"] = ps_big.tile(
                            [128, 1024], F32, name="bigp", tag="big"
                        )
                    big = state["big"]
                    for c in range(cs, ce):
                        nc.tensor.matmul(
                            big[0:64, 0:512],
                            lhsT=wk_sb[:, c, HD : 2 * HD],
                            rhs=xp[c][:, c0:c1],
                            start=(c == 0),
                            stop=(c == 5),
                            skip_group_check=True,
                        )
                    if copy:
                        nc.vector.tensor_copy(
                            out=kt[64:128, c0:c1], in_=big[0:64, 0:512]
                        )

                return go

            def v_piece(j0, j1, copy):
                def go():
                    if "bigv" not in state:
                        state["bigv"] = ps_big.tile(
                            [128, 1024], F32, name="bigpv", tag="big"
                        )
                    bigv = state["bigv"]
                    for j in range(j0, j1):
                        t0 = c0 + 128 * j
                        for c in range(6):
                            nc.tensor.matmul(
                                bigv[:, 64 * j : 64 * (j + 1)],
                                lhsT=xp[c][:, t0 : t0 + 128],
                                rhs=wv_sb[:, c, HD : 2 * HD],
                                start=(c == 0),
                                stop=(c == 5),
                                skip_group_check=True,
                            )
                    if copy:
                        nc.vector.tensor_copy(
                            out=vB[:, 4 * p : 4 * p + 4, 0:HD],
                            in_=bigv[:, 0:256].rearrange("p (j d) -> p j d", j=4),
                        )

                return go

            return [
                k_piece(0, 3, False),
                k_piece(3, 6, True),
                v_piece(0, 2, False),
                v_piece(2, 4, True),
            ]

        # attention tasks: one per (block b, slot, k-group)
        def emit_scores(t):
            b, slot, kbs, _, _ = t
            r0, r1 = (0, 64) if slot == 0 else (64, 128)
            gw = 256 * len(kbs)
            st = ps_big.tile([128, 1024], F32, name="st", tag="big")
            for j, kb in enumerate(kbs):
                nc.tensor.matmul(
                    st[:, 256 * j : 256 * (j + 1)],
                    lhsT=kt[r0:r1, 128 * kb : 128 * (kb + 1)],
                    rhs=qt[r0:r1, QB * b : QB * (b + 1)],
                    start=True,
                    stop=True,
                )
            pt = ptpool.tile([128, 1024], BF16, name="pt", tag="pt")
            nc.scalar.activation(out=pt[:, 0:gw], in_=st[:, 0:gw], func=EXP, scale=scale)
            if t[3]:  # last group: diagonal causal masks
                nd = len(kbs)
                if slot == 0:
                    # phys diag blocks 2b (keep q>=k: col>=p) and 2b+1
                    # (keep col >= 128+p), in place on gpsimd
                    nc.gpsimd.affine_select(
                        out=pt[:, 256 * (nd - 2) : 256 * (nd - 1)],
                        in_=pt[:, 256 * (nd - 2) : 256 * (nd - 1)],
                        compare_op=GE,
                        fill=0.0,
                        base=0,
                        channel_multiplier=-1,
                        pattern=[[1, QB]],
                    )
                    nc.gpsimd.affine_select(
                        out=pt[:, 256 * (nd - 1) : 256 * nd],
                        in_=pt[:, 256 * (nd - 1) : 256 * nd],
                        compare_op=GE,
                        fill=0.0,
                        base=-128,
                        channel_multiplier=-1,
                        pattern=[[1, QB]],
                    )
                else:
                    # logical diag block b: host-supplied parity mask
                    nc.gpsimd.tensor_mul(
                        out=pt[:, 256 * (nd - 1) : 256 * nd],
                        in0=pt[:, 256 * (nd - 1) : 256 * nd],
                        in1=maskb,
                    )
            return pt

        def emit_pv(t, pt, y):
            """y[q, d] += P^T[k, q].T @ V[k, d]; col 64 = softmax denom."""
            b, slot, kbs, last, first = t
            v = vA if slot == 0 else vB
            nlast = (2 * b + 1) if slot == 0 else b
            base = 256 * slot
            for j, kb in enumerate(kbs):
                for h in range(2):
                    nc.tensor.matmul(
                        y[:, base + 128 * h : base + 128 * h + HD + 1],
                        lhsT=pt[:, 256 * j + 128 * h : 256 * j + 128 * (h + 1)],
                        rhs=v[:, kb, :],
                        start=(kb == 0),
                        stop=(kb == nlast),
                        skip_group_check=True,
                    )

        def emit_finalize_a(b, y):
            """normalize y (head A) into y_sb columns for q-tiles 2b, 2b+1."""
            for h in range(2):
                r = rpool.tile([128, 1], F32, name="r", tag="r")
                with nc.allow_low_precision(reason="softmax denom recip"):
                    nc.vector.reciprocal(
                        out=r, in_=y[:, 128 * h + HD : 128 * h + HD + 1]
                    )
                nc.vector.tensor_scalar_mul(
                    out=y_sb[:, (2 * b + h) * HD : (2 * b + h + 1) * HD],
                    in0=y[:, 128 * h : 128 * h + HD],
                    scalar1=r,
                )

        def emit_finalize_b(b, y):
            """ship raw y+denom (head B partial) to DRAM."""
            yb = rpool.tile([128, 2 * (HD + 1)], BF16, name="yb", tag="yb", bufs=3)
            nc.vector.tensor_copy(
                out=yb.rearrange("p (i d) -> p i d", i=2),
                in_=y[:, 256:512].rearrange("p (i d) -> p i d", i=2)[
                    :, :, 0 : HD + 1
                ],
            )
            nc.sync.dma_start(
                out=yb_d.ap()[QB * b : QB * (b + 1), :].rearrange(
                    "(i p) d -> p i d", p=128
                ),
                in_=yb.rearrange("p (i d) -> p i d", i=2),
            )

        def emit_out_block(b):
            """transpose normalized y (head A) and project: rows 256b..+256."""
            ytp = ps_big.tile([HD, 256], BF16, name="ytp", tag="big")
            for h in range(2):
                nc.tensor.transpose(
                    ytp[:, 128 * h : 128 * (h + 1)],
                    y_sb[:, (2 * b + h) * HD : (2 * b + h + 1) * HD],
                    ident,
                )
            yts = rpool.tile([HD, 256], BF16, name="yts", tag="yts", bufs=3)
            nc.vector.tensor_copy(out=yts, in_=ytp)
            for h in range(2):
                po = ps_big.tile([128, 1024], F32, name="po", tag="big")
                for c0, c1 in ((0, 512), (512, 768)):
                    nc.tensor.matmul(
                        po[:, c0:c1],
                        lhsT=yts[:, 128 * h : 128 * (h + 1)],
                        rhs=wp_sb[:, c0:c1],
                        start=True,
                        stop=True,
                    )
                posb = opool.tile([128, C], BF16, name="posb", tag="po")
                nc.vector.tensor_copy(out=posb, in_=po[:, 0:C])
                r0 = QB * b + 128 * h
                nc.sync.dma_start(out=out_d.ap()[r0 : r0 + 128, :], in_=posb)

        # ---- projection pieces, drained between attention tasks ----
        # qk/k pieces gate score emission; v pieces only gate PV retires
        chains = []
        chain_idx = {}
        v_chains = []
        v_idx = {}
        for sl in range(NTS):
            mp = mk_main_pieces(sl)
            chains.extend(mp[0:2])
            v_chains.extend(mp[2:4])
            chain_idx[f"m{sl}"] = len(chains)
            v_idx[f"m{sl}"] = len(v_chains)
            if sl % 2 == 0:
                p = sl // 2
                pp = mk_packed_pieces(p)
                chains.extend(pp[0:2])
                v_chains.extend(pp[2:4])
                chain_idx[f"p{p}"] = len(chains)
                v_idx[f"p{p}"] = len(v_chains)
        chains_done = 0
        v_done = 0

        def drain_chains(upto):
            nonlocal chains_done
            while chains_done < upto:
                chains[chains_done]()
                chains_done += 1

        def drain_v(upto):
            nonlocal v_done
            while v_done < upto:
                v_chains[v_done]()
                v_done += 1

        # ---- task stream: A-units in block order, B-units lagged 2 blocks ----
        tasks = []  # (b, slot, kbs, is_last_group, is_first_group)

        def push_unit(b, slot, nkb):
            gs = _groups(nkb)
            for gi, kbs in enumerate(gs):
                tasks.append((b, slot, kbs, gi == len(gs) - 1, gi == 0))

        for b in range(NQB):
            push_unit(b, 0, 2 * b + 2)
            if b >= 2:
                push_unit(b - 2, 1, b - 1)
        push_unit(NQB - 2, 1, NQB - 1)
        push_unit(NQB - 1, 1, NQB)

        LAG = 6
        pending = collections.deque()  # (task, pt, y), PV emitted LAG tasks later
        y_of = {}  # block -> y tile (A at [0:256]; B of block b-2 at [256:512])

        def retire(p):
            rb, rslot = p[0][0], p[0][1]
            if rslot == 0:
                drain_v(v_idx[f"m{min(NTS - 1, (2 * rb + 1) // 4)}"])
            else:
                drain_v(v_idx[f"p{rb // 4}"])
            emit_pv(*p)
            pb, pslot, _, plast, _ = p[0]
            if plast:
                # defer finalize: its first DVE op waits on this unit's last
                # PV; emitting it later keeps the DVE queue head unblocked
                if pslot == 0:

                    def fin_a(pb=pb, y=p[2]):
                        emit_finalize_a(pb, y)
                        work_q.append(lambda: emit_out_block(pb))

                    work_q.append(fin_a)
                else:
                    work_q.append(lambda pb=pb, y=p[2]: emit_finalize_b(pb, y))

        for ti, t in enumerate(tasks):
            b, slot, kbs, last, first = t
            # loose (ready) work FIRST: the PE is in-order, so anything
            # emitted after a stall-prone instruction would stall with it.
            if len(pending) > LAG - 1:
                retire(pending.popleft())
            # proportional voluntary chain pacing over the first ~60% of tasks
            target = min(len(chains), 1 + (ti * len(chains)) // int(0.6 * len(tasks)))
            if chains_done < target:
                drain_chains(chains_done + 1)
            elif work_q:
                work_q.popleft()()
            elif v_done < len(v_chains):
                drain_v(v_done + 1)

            # forced proj deadlines for this task's data
            if slot == 0:
                drain_chains(chain_idx[f"m{min(NTS - 1, (2 * b + 1) // 4)}"])
            else:
                drain_chains(chain_idx[f"p{b // 4}"])
            pt = emit_scores(t)
            # A(b) owns tile y_of[b] rows [0:256]; B(b) shares A(b+2)'s tile
            # at rows [256:512] (A(b+2) is emitted just before B(b))
            ykey = b if slot == 0 else b + 2
            if first and ykey not in y_of:
                y_of[ykey] = ps_y.tile([128, 512], F32, name="y", tag="y")
            y = y_of[ykey]
            pending.append((t, pt, y))
        while pending:
            retire(pending.popleft())
        while work_q:
            work_q.popleft()()

    nc.compile()
    return nc


def _get_nc():
    if "nc" not in _CACHE:
        _CACHE["nc"] = _build_nc()
    return _CACHE["nc"]


def _core_inputs(x, w_attn, w_proj):
    """Per-core input dicts (bf16, host-side transpose + parity packing)."""
    import ml_dtypes

    bf16 = ml_dtypes.bfloat16
    x = np.asarray(x, dtype=np.float32).reshape(T, C)
    w_attn = np.asarray(w_attn, dtype=np.float32)
    w_proj = np.asarray(w_proj, dtype=np.float32)

    xt = np.ascontiguousarray(x.T).astype(bf16)  # [C, T]
    xt_blocks = xt.reshape(C, NKB, KB)
    # parity-packed x^T: even k-blocks (cores 0-3) / odd (cores 4-7)
    xp_even = np.ascontiguousarray(
        xt_blocks[:, 0::2, :].reshape(C, T // 2)
    ).astype(bf16)
    xp_odd = np.ascontiguousarray(
        xt_blocks[:, 1::2, :].reshape(C, T // 2)
    ).astype(bf16)

    # parity diag masks [KB, QB]: even keeps col>=p, odd keeps col>=128+p
    p = np.arange(KB)[:, None]
    col = np.arange(QB)[None, :]
    mask_even = (col >= p).astype(bf16)
    mask_odd = (col >= p + 128).astype(bf16)

    in_maps = []
    for c in range(N_CORES):
        hA = c
        hB = 8 + (c % 4)
        parity = 0 if c < 4 else 1

        def cols(w, h):
            return w[:, h * HD : (h + 1) * HD]

        wq = np.concatenate(
            [cols(w_attn[:, 0:C], hA), cols(w_attn[:, 0:C], hB)], axis=1
        ).astype(bf16)
        wk = np.concatenate(
            [cols(w_attn[:, C : 2 * C], hA), cols(w_attn[:, C : 2 * C], hB)], axis=1
        ).astype(bf16)
        wv = np.concatenate(
            [cols(w_attn[:, 2 * C : 3 * C], hA), cols(w_attn[:, 2 * C : 3 * C], hB)],
            axis=1,
        ).astype(bf16)
        wp = np.ascontiguousarray(w_proj[hA * HD : (hA + 1) * HD, :]).astype(bf16)
        in_maps.append(
            {
                "xt": xt,
                "xp": xp_even if parity == 0 else xp_odd,
                "wq": np.ascontiguousarray(wq),
                "wk": np.ascontiguousarray(wk),
                "wv": np.ascontiguousarray(wv),
                "wp": wp,
                "maskb": mask_even if parity == 0 else mask_odd,
            }
        )
    return in_maps


def _get_runner():
    """Build the shard_map'd PJRT executable once and reuse it across calls."""
    if "runner" in _CACHE:
        return _CACHE["runner"]
    import jax
    import concourse.mybir as mybir
    from concourse import bass2jax
    from jax.experimental.shard_map import shard_map
    from jax.sharding import Mesh, PartitionSpec

    nc = _get_nc()
    bass2jax.install_neuronx_cc_hook()

    in_names, out_names, out_avals, zero_outs = [], [], [], []
    for alloc in nc.m.functions[0].allocations:
        if not isinstance(alloc, mybir.MemoryLocationSet):
            continue
        name = alloc.memorylocations[0].name
        if alloc.kind == "ExternalInput":
            if nc.partition_id_tensor and name == nc.partition_id_tensor.name:
                continue
            in_names.append(name)
        elif alloc.kind == "ExternalOutput":
            shape = tuple(alloc.tensor_shape)
            dtype = mybir.dt.np(alloc.dtype)
            out_names.append(name)
            out_avals.append(jax.core.ShapedArray(shape, dtype))
            zero_outs.append(np.zeros(shape, dtype))
    n_params = len(in_names)
    all_in_names = in_names + out_names
    if nc.partition_id_tensor:
        all_in_names = all_in_names + [nc.partition_id_tensor.name]

    def _body(*args):
        operands = list(args)
        if nc.partition_id_tensor:
            operands.append(bass2jax.partition_id_tensor())
        outs = bass2jax._bass_exec_p.bind(
            *operands,
            out_avals=tuple(out_avals),
            in_names=tuple(all_in_names),
            out_names=tuple(out_names),
            lowering_input_output_aliases=(),
            sim_require_finite=True,
            sim_require_nnan=True,
            nc=nc,
        )
        return tuple(outs)

    devices = jax.devices()[:N_CORES]
    mesh = Mesh(np.asarray(devices), ("core",))
    n_out = len(out_names)
    donate = tuple(range(n_params, n_params + n_out))
    sharded = jax.jit(
        shard_map(
            _body,
            mesh=mesh,
            in_specs=(PartitionSpec("core"),) * (n_params + n_out),
            out_specs=(PartitionSpec("core"),) * n_out,
            check_rep=False,
        ),
        donate_argnums=donate,
        keep_unused=True,
    )

    def run(in_maps):
        concat_in = [
            np.concatenate([in_maps[c][name] for c in range(N_CORES)], axis=0)
            for name in in_names
        ]
        concat_zeros = [
            np.zeros((N_CORES * z.shape[0], *z.shape[1:]), z.dtype)
            for z in zero_outs
        ]
        out_arrs = sharded(*concat_in, *concat_zeros)
        return [
            {
                name: np.asarray(out_arrs[i]).reshape(
                    N_CORES, *out_avals[i].shape
                )[c]
                for i, name in enumerate(out_names)
            }
            for c in range(N_CORES)
        ]

    _CACHE["runner"] = run
    return run


def kernel(x, w_attn, w_proj):
    run = _get_runner()
    w_proj_f32 = np.asarray(w_proj, dtype=np.float32)
    in_maps = _core_inputs(np.asarray(x), np.asarray(w_attn), w_proj_f32)
    results = run(in_maps)

    out = np.zeros((T, C), dtype=np.float32)
    for c in range(N_CORES):
        out += results[c]["out"].astype(np.float32)

    # heads 8-11: combine parity partials, then project on host (fp32)
    Y = np.empty((T, 4 * HD), dtype=np.float32)
    for j in range(4):
        e = results[j]["yb"].astype(np.float32)
        o = results[4 + j]["yb"].astype(np.float32)
        num = e[:, 0:HD] + o[:, 0:HD]
        den = e[:, HD : HD + 1] + o[:, HD : HD + 1]
        Y[:, j * HD : (j + 1) * HD] = num / den
    out += Y @ w_proj_f32[8 * HD : 12 * HD, :]
    return out.reshape(1, T, C)


# revision 26
# speedup vs baseline: 1.0286x; 1.0286x over previous
"""Causal self-attention (B=1, T=4096, C=768, H=12) on 8 Trainium2 NeuronCores.

Sharding (uniform SPMD program, zero dummy work):
  - heads 0-7 live whole on cores 0-7 ("slot A", full softmax on device)
  - heads 8-11 are split between core pairs (c, c+4) by k-block PARITY:
    core c<4 handles even 128-row k-blocks, core c+4 the odd ones, each
    producing an UNNORMALIZED partial y plus the partial softmax
    denominator ("slot B"). The host adds the two partials and divides --
    exact because no max-subtraction is used (scores are small).
    The parity selection is pure data: the host packs x^T's k-blocks of
    the matching parity contiguously (xp), so both program variants are
    the same instruction stream.

Per core the device program:
  1. loads host-pretransposed x^T (bf16) -- no on-chip transposes of x
  2. projects Q^T/K^T (heads on partitions) and V in natural [t, d]
     layout (direct, no V transpose)
  3. causal attention per 256-row q-block: S^T = K^T.T Q^T per 128-wide
     k-block, exp on the scalar engine in 1536-wide batches, diagonal
     masks on gpsimd, then P^T.T V accumulated as y[q, d] with an extra
     ones-column in V giving the softmax denominator in column 64
  4. slot A: normalize y, transpose 128x64 tiles via the PE array, and
     project with this head's 64 rows of w_proj; slot B: ship raw y+denom
  The host sums the 8 partial projections and adds heads 8-11's
  contribution (a small [4096,256]x[256,768] matmul) in fp32.

All matmul inputs are bf16 (fp32 PSUM accumulation); the relative error
vs the fp32 reference stays ~1e-3, well inside the 2e-2 gate.
"""

import sys

sys.path.insert(0, "/opt/trn_rl_repo")

import numpy as np

T = 4096
C = 768
H = 12
HD = 64
N_CORES = 8
TS = 512  # t-slice for x load / projection
NTS = T // TS  # 8
QB = 256  # q-block rows
NQB = T // QB  # 16
KB = 128  # k-block rows
NKB = T // KB  # 32
GRP = 4  # k-blocks per score/exp group (4*256 = 1024 wide = 2 PSUM banks)

_CACHE = {}


def _groups(n):
    """Chunk k-block indices 0..n-1 into groups of GRP (ascending)."""
    return [list(range(g, min(g + GRP, n))) for g in range(0, n, GRP)]


def _build_nc():
    import concourse.bacc as bacc
    import concourse.tile as tile
    import concourse.mybir as mybir
    from concourse.masks import make_identity
    from contextlib import ExitStack
    import collections

    F32 = mybir.dt.float32
    BF16 = mybir.dt.bfloat16
    EXP = mybir.ActivationFunctionType.Exp
    GE = mybir.AluOpType.is_ge

    nc = bacc.Bacc(
        "TRN2",
        target_bir_lowering=False,
        debug=False,
        enable_asserts=True,
        num_devices=N_CORES,
    )
    # host-pretransposed x^T, full [C, T]
    xt_d = nc.dram_tensor("xt", [C, T], BF16, kind="ExternalInput")
    # x^T with only this core's parity of k-blocks, packed [C, T//2]
    xp_d = nc.dram_tensor("xp", [C, T // 2], BF16, kind="ExternalInput")
    wq_d = nc.dram_tensor("wq", [C, 2 * HD], BF16, kind="ExternalInput")
    wk_d = nc.dram_tensor("wk", [C, 2 * HD], BF16, kind="ExternalInput")
    wv_d = nc.dram_tensor("wv", [C, 2 * HD], BF16, kind="ExternalInput")
    wp_d = nc.dram_tensor("wp", [HD, C], BF16, kind="ExternalInput")
    mb_d = nc.dram_tensor("maskb", [KB, QB], BF16, kind="ExternalInput")
    out_d = nc.dram_tensor("out", [T, C], BF16, kind="ExternalOutput")
    yb_d = nc.dram_tensor("yb", [HD + 1, T], BF16, kind="ExternalOutput")

    scale = 1.0 / float(np.sqrt(HD))

    with ExitStack() as ctx:
        tc = ctx.enter_context(tile.TileContext(nc))
        singles = ctx.enter_context(tc.tile_pool(name="singles", bufs=1))
        ptpool = ctx.enter_context(tc.tile_pool(name="ptpool", bufs=8))
        opool = ctx.enter_context(tc.tile_pool(name="opool", bufs=2))
        rpool = ctx.enter_context(tc.tile_pool(name="rpool", bufs=4))
        ps_big = ctx.enter_context(tc.tile_pool(name="ps_big", bufs=3, space="PSUM"))
        ps_y = ctx.enter_context(tc.tile_pool(name="ps_y", bufs=2, space="PSUM"))

        # ---- persistent SBUF tensors ----
        xt = [singles.tile([128, T], BF16, name=f"xt{c}") for c in range(6)]
        xp = [singles.tile([128, T // 2], BF16, name=f"xp{c}") for c in range(6)]
        qt = singles.tile([128, T], BF16)  # rows 0:64 head A, 64:128 head B
        kt = singles.tile([128, T], BF16)  # B rows use cols 0:T//2 (packed)
        vA = singles.tile([128, NKB, HD + 1], BF16)
        vB = singles.tile([128, NKB // 2, HD + 1], BF16)
        wq_sb = singles.tile([128, 6, 2 * HD], BF16)
        wkv_sb = singles.tile([128, 6, 2 * HD], BF16)  # [wk_A | wv_A]
        wkvB_sb = singles.tile([128, 6, 2 * HD], BF16)  # [wk_B | wv_B]
        wp_sb = singles.tile([HD, C], BF16)
        maskb = singles.tile([KB, QB], BF16)
        ident = singles.tile([128, 128], BF16)

        ident_f32 = singles.tile([128, 128], F32)
        make_identity(nc, ident_f32)
        nc.vector.tensor_copy(out=ident, in_=ident_f32)
        nc.gpsimd.memset(vA[:, :, HD : HD + 1], 1.0)
        nc.gpsimd.memset(vB[:, :, HD : HD + 1], 1.0)

        # ---- weight + mask DMAs (vector-engine queue; tiny) ----
        nc.scalar.dma_start(
            out=wq_sb, in_=wq_d.ap().rearrange("(c p) j -> p c j", p=128)
        )
        nc.scalar.dma_start(
            out=wkv_sb, in_=wk_d.ap().rearrange("(c p) j -> p c j", p=128)
        )
        nc.scalar.dma_start(
            out=wkvB_sb, in_=wv_d.ap().rearrange("(c p) j -> p c j", p=128)
        )
        nc.scalar.dma_start(out=wp_sb, in_=wp_d.ap())
        nc.scalar.dma_start(out=maskb, in_=mb_d.ap())

        # ---- x^T input DMAs (sync-engine queue), finest slices first ----
        for c in range(6):
            nc.sync.dma_start(
                out=xt[c][:, 0:TS], in_=xt_d.ap()[128 * c : 128 * (c + 1), 0:TS]
            )
        for c in range(6):
            nc.sync.dma_start(
                out=xp[c][:, 0:1024], in_=xp_d.ap()[128 * c : 128 * (c + 1), 0:1024]
            )
        for c in range(6):
            nc.sync.dma_start(
                out=xt[c][:, TS : 2 * TS],
                in_=xt_d.ap()[128 * c : 128 * (c + 1), TS : 2 * TS],
            )
        for c in range(6):
            nc.sync.dma_start(
                out=xt[c][:, 1024:2048],
                in_=xt_d.ap()[128 * c : 128 * (c + 1), 1024:2048],
            )
        for c in range(6):
            nc.sync.dma_start(
                out=xt[c][:, 2048:3072],
                in_=xt_d.ap()[128 * c : 128 * (c + 1), 2048:3072],
            )
        for c in range(6):
            nc.sync.dma_start(
                out=xp[c][:, 1024:2048],
                in_=xp_d.ap()[128 * c : 128 * (c + 1), 1024:2048],
            )
        for c in range(6):
            nc.sync.dma_start(
                out=xt[c][:, 3072:4096],
                in_=xt_d.ap()[128 * c : 128 * (c + 1), 3072:4096],
            )

        # ---- emission helpers ----
        work_q = collections.deque()  # deferred closures to fill PE gaps

        # projection emitted as small "pieces" (<=6 matmuls) so interleaved
        # attention score groups are never stuck behind a long proj burst in
        # PE program order. Pieces of one logical chain share a PSUM tile via
        # the shared dict; skip_group_check allows interleaved accumulation.
        def mk_main_pieces(s):
            c0, c1 = s * TS, (s + 1) * TS
            state = {}

            def qkv_piece(cs, ce, copy):
                # one [128, 1024] tile: q at [0:512]; [K^T_A; V^T_A] stacked
                # on partitions at [512:1024] via merged [wk|wv] stationary
                def go():
                    if "big" not in state:
                        state["big"] = ps_big.tile(
                            [128, 1024], F32, name="big", tag="big"
                        )
                    big = state["big"]
                    for c in range(cs, ce):
                        nc.tensor.matmul(
                            big[:, 0:512],
                            lhsT=wq_sb[:, c, :],
                            rhs=xt[c][:, c0:c1],
                            start=(c == 0),
                            stop=(c == 5),
                            skip_group_check=True,
                        )
                        nc.tensor.matmul(
                            big[:, 512:1024],
                            lhsT=wkv_sb[:, c, :],
                            rhs=xt[c][:, c0:c1],
                            start=(c == 0),
                            stop=(c == 5),
                            skip_group_check=True,
                        )
                    if copy:
                        nc.vector.tensor_copy(out=qt[:, c0:c1], in_=big[:, 0:512])
                        nc.vector.tensor_copy(
                            out=kt[0:64, c0:c1], in_=big[0:64, 512:1024]
                        )
                        vt = rpool.tile([HD, TS], BF16, name="vt", tag="vt", bufs=3)
                        nc.vector.tensor_copy(out=vt, in_=big[64:128, 512:1024])
                        state["vt"] = vt

                return go

            def v_piece(j0, j1, copy):
                # transpose V^T -> v_nat[k, d] per 128-col k-block
                def go():
                    vt = state["vt"]
                    vtp = ps_big.tile([128, 256], BF16, name="vtp", tag="big")
                    for j in range(j0, j1):
                        nc.tensor.transpose(
                            vtp[:, 128 * (j - j0) : 128 * (j - j0 + 1)][:, 0:HD],
                            vt[:, 128 * j : 128 * (j + 1)],
                            ident,
                        )
                    nc.vector.tensor_copy(
                        out=vA[:, 4 * s + j0 : 4 * s + j1, 0:HD],
                        in_=vtp[:, 0:256].rearrange("p (j d) -> p j d", j=2)[
                            :, :, 0:HD
                        ],
                    )

                return go

            return [
                qkv_piece(0, 3, False),
                qkv_piece(3, 6, True),
                v_piece(0, 2, False),
                v_piece(2, 4, True),
            ]

        def mk_packed_pieces(p):
            c0, c1 = p * TS, (p + 1) * TS
            state = {}

            def kv_piece(cs, ce, copy):
                def go():
                    if "big" not in state:
                        state["big"] = ps_big.tile(
                            [128, 1024], F32, name="bigp", tag="big"
                        )
                    big = state["big"]
                    for c in range(cs, ce):
                        nc.tensor.matmul(
                            big[:, 0:512],
                            lhsT=wkv_sb[:, c, HD : 2 * HD].rearrange(
                                "p d -> p d"
                            )
                            if False
                            else wkvB_sb[:, c, :],
                            rhs=xp[c][:, c0:c1],
                            start=(c == 0),
                            stop=(c == 5),
                            skip_group_check=True,
                        )
                    if copy:
                        nc.vector.tensor_copy(
                            out=kt[64:128, c0:c1], in_=big[0:64, 0:512]
                        )
                        vt = rpool.tile([HD, TS], BF16, name="vtb", tag="vt", bufs=3)
                        nc.vector.tensor_copy(out=vt, in_=big[64:128, 0:512])
                        state["vt"] = vt

                return go

            def v_piece(j0, j1, copy):
                def go():
                    vt = state["vt"]
                    vtp = ps_big.tile([128, 256], BF16, name="vtpb", tag="big")
                    for j in range(j0, j1):
                        nc.tensor.transpose(
                            vtp[:, 128 * (j - j0) : 128 * (j - j0 + 1)][:, 0:HD],
                            vt[:, 128 * j : 128 * (j + 1)],
                            ident,
                        )
                    nc.vector.tensor_copy(
                        out=vB[:, 4 * p + j0 : 4 * p + j1, 0:HD],
                        in_=vtp[:, 0:256].rearrange("p (j d) -> p j d", j=2)[
                            :, :, 0:HD
                        ],
                    )

                return go

            return [
                kv_piece(0, 3, False),
                kv_piece(3, 6, True),
                v_piece(0, 2, False),
                v_piece(2, 4, True),
            ]

        # attention tasks: one per (block b, slot, k-group)
        def emit_scores(t):
            b, slot, kbs, _, _ = t
            r0, r1 = (0, 64) if slot == 0 else (64, 128)
            gw = 256 * len(kbs)
            st = ps_big.tile([128, 1024], F32, name="st", tag="big")
            for j, kb in enumerate(kbs):
                nc.tensor.matmul(
                    st[:, 256 * j : 256 * (j + 1)],
                    lhsT=kt[r0:r1, 128 * kb : 128 * (kb + 1)],
                    rhs=qt[r0:r1, QB * b : QB * (b + 1)],
                    start=True,
                    stop=True,
                )
            pt = ptpool.tile([128, 1024], BF16, name="pt", tag="pt")
            nc.scalar.activation(out=pt[:, 0:gw], in_=st[:, 0:gw], func=EXP, scale=scale)
            if t[3]:  # last group: diagonal causal masks
                nd = len(kbs)
                if slot == 0:
                    # phys diag blocks 2b (keep q>=k: col>=p) and 2b+1
                    # (keep col >= 128+p), in place on gpsimd
                    nc.gpsimd.affine_select(
                        out=pt[:, 256 * (nd - 2) : 256 * (nd - 1)],
                        in_=pt[:, 256 * (nd - 2) : 256 * (nd - 1)],
                        compare_op=GE,
                        fill=0.0,
                        base=0,
                        channel_multiplier=-1,
                        pattern=[[1, QB]],
                    )
                    nc.gpsimd.affine_select(
                        out=pt[:, 256 * (nd - 1) : 256 * nd],
                        in_=pt[:, 256 * (nd - 1) : 256 * nd],
                        compare_op=GE,
                        fill=0.0,
                        base=-128,
                        channel_multiplier=-1,
                        pattern=[[1, QB]],
                    )
                else:
                    # logical diag block b: host-supplied parity mask
                    nc.gpsimd.tensor_mul(
                        out=pt[:, 256 * (nd - 1) : 256 * nd],
                        in0=pt[:, 256 * (nd - 1) : 256 * nd],
                        in1=maskb,
                    )
            return pt

        def emit_pv(t, pt, y):
            """y^T[d, q] += V[k, d].T @ P^T[k, q]; row 64 = softmax denom."""
            b, slot, kbs, last, first = t
            v = vA if slot == 0 else vB
            nlast = (2 * b + 1) if slot == 0 else b
            base = 256 * slot
            for j, kb in enumerate(kbs):
                nc.tensor.matmul(
                    y[0 : HD + 1, base : base + 256],
                    lhsT=v[:, kb, :],
                    rhs=pt[:, 256 * j : 256 * (j + 1)],
                    start=(kb == 0),
                    stop=(kb == nlast),
                    skip_group_check=True,
                )

        def emit_finalize_a(b, y):
            """normalize y^T (head A): divide rows 0:64 by denom row 64."""
            r = rpool.tile([1, QB], F32, name="r", tag="r")
            with nc.allow_low_precision(reason="softmax denom recip"):
                nc.vector.reciprocal(out=r, in_=y[HD : HD + 1, 0:QB])
            bc = rpool.tile([HD, QB], F32, name="bc", tag="bc", bufs=3)
            nc.gpsimd.partition_broadcast(bc, r)
            yn = rpool.tile([HD, QB], BF16, name="yn", tag="yn", bufs=4)
            nc.vector.tensor_mul(out=yn, in0=y[0:HD, 0:QB], in1=bc)
            return yn

        def emit_finalize_b(b, y):
            """ship raw y^T+denom (head B partial) to DRAM, [65, T] layout."""
            yb = rpool.tile([HD + 1, QB], BF16, name="yb", tag="yb", bufs=2)
            nc.vector.tensor_copy(out=yb, in_=y[0 : HD + 1, QB : 2 * QB])
            nc.sync.dma_start(
                out=yb_d.ap()[:, QB * b : QB * (b + 1)], in_=yb
            )

        def emit_out_block(b, yn):
            """project normalized y^T (head A): out rows 256b..+256."""
            for h in range(2):
                po = ps_big.tile([128, 1024], F32, name="po", tag="big")
                for c0, c1 in ((0, 512), (512, 768)):
                    nc.tensor.matmul(
                        po[:, c0:c1],
                        lhsT=yn[:, 128 * h : 128 * (h + 1)],
                        rhs=wp_sb[:, c0:c1],
                        start=True,
                        stop=True,
                    )
                posb = opool.tile([128, C], BF16, name="posb", tag="po")
                nc.vector.tensor_copy(out=posb, in_=po[:, 0:C])
                r0 = QB * b + 128 * h
                nc.sync.dma_start(out=out_d.ap()[r0 : r0 + 128, :], in_=posb)

        # ---- projection pieces, drained between attention tasks ----
        # qk/k pieces gate score emission; v pieces only gate PV retires
        chains = []
        chain_idx = {}
        v_chains = []
        v_idx = {}
        for sl in range(NTS):
            mp = mk_main_pieces(sl)
            chains.extend(mp[0:2])
            v_chains.extend(mp[2:4])
            chain_idx[f"m{sl}"] = len(chains)
            v_idx[f"m{sl}"] = len(v_chains)
            if sl % 2 == 0:
                p = sl // 2
                pp = mk_packed_pieces(p)
                chains.extend(pp[0:2])
                v_chains.extend(pp[2:4])
                chain_idx[f"p{p}"] = len(chains)
                v_idx[f"p{p}"] = len(v_chains)
        chains_done = 0
        v_done = 0

        def drain_chains(upto):
            nonlocal chains_done
            while chains_done < upto:
                chains[chains_done]()
                chains_done += 1

        def drain_v(upto):
            nonlocal v_done
            while v_done < upto:
                v_chains[v_done]()
                v_done += 1

        # ---- task stream: A-units in block order, B-units lagged 2 blocks ----
        tasks = []  # (b, slot, kbs, is_last_group, is_first_group)

        def push_unit(b, slot, nkb):
            gs = _groups(nkb)
            for gi, kbs in enumerate(gs):
                tasks.append((b, slot, kbs, gi == len(gs) - 1, gi == 0))

        for b in range(NQB):
            push_unit(b, 0, 2 * b + 2)
            if b >= 2:
                push_unit(b - 2, 1, b - 1)
        push_unit(NQB - 2, 1, NQB - 1)
        push_unit(NQB - 1, 1, NQB)

        LAG = 6
        pending = collections.deque()  # (task, pt, y), PV emitted LAG tasks later
        y_of = {}  # block -> y tile (A at [0:256]; B of block b-2 at [256:512])

        def retire(p):
            rb, rslot = p[0][0], p[0][1]
            if rslot == 0:
                drain_v(v_idx[f"m{min(NTS - 1, (2 * rb + 1) // 4)}"])
            else:
                drain_v(v_idx[f"p{rb // 4}"])
            emit_pv(*p)
            pb, pslot, _, plast, _ = p[0]
            if plast:
                # defer finalize: its first DVE op waits on this unit's last
                # PV; emitting it later keeps the DVE queue head unblocked
                if pslot == 0:

                    def fin_a(pb=pb, y=p[2]):
                        yn = emit_finalize_a(pb, y)
                        work_q.append(lambda: emit_out_block(pb, yn))

                    work_q.append(fin_a)
                else:
                    work_q.append(lambda pb=pb, y=p[2]: emit_finalize_b(pb, y))

        for ti, t in enumerate(tasks):
            b, slot, kbs, last, first = t
            # loose (ready) work FIRST: the PE is in-order, so anything
            # emitted after a stall-prone instruction would stall with it.
            if len(pending) > LAG - 1:
                retire(pending.popleft())
            # proportional voluntary chain pacing over the first ~60% of tasks
            target = min(len(chains), 1 + (ti * len(chains)) // int(0.6 * len(tasks)))
            if chains_done < target:
                drain_chains(chains_done + 1)
            elif work_q:
                work_q.popleft()()
            elif v_done < len(v_chains):
                drain_v(v_done + 1)

            # forced proj deadlines for this task's data
            if slot == 0:
                drain_chains(chain_idx[f"m{min(NTS - 1, (2 * b + 1) // 4)}"])
            else:
                drain_chains(chain_idx[f"p{b // 4}"])
            pt = emit_scores(t)
            # A(b) owns tile y_of[b] rows [0:256]; B(b) shares A(b+2)'s tile
            # at rows [256:512] (A(b+2) is emitted just before B(b))
            ykey = b if slot == 0 else b + 2
            if first and ykey not in y_of:
                y_of[ykey] = ps_y.tile([128, 512], F32, name="y", tag="y")
            y = y_of[ykey]
            pending.append((t, pt, y))
        while pending:
            retire(pending.popleft())
        while work_q:
            work_q.popleft()()

    nc.compile()
    return nc


def _get_nc():
    if "nc" not in _CACHE:
        _CACHE["nc"] = _build_nc()
    return _CACHE["nc"]


def _core_inputs(x, w_attn, w_proj):
    """Per-core input dicts (bf16, host-side transpose + parity packing)."""
    import ml_dtypes

    bf16 = ml_dtypes.bfloat16
    x = np.asarray(x, dtype=np.float32).reshape(T, C)
    w_attn = np.asarray(w_attn, dtype=np.float32)
    w_proj = np.asarray(w_proj, dtype=np.float32)

    xt = np.ascontiguousarray(x.T).astype(bf16)  # [C, T]
    xt_blocks = xt.reshape(C, NKB, KB)
    # parity-packed x^T: even k-blocks (cores 0-3) / odd (cores 4-7)
    xp_even = np.ascontiguousarray(
        xt_blocks[:, 0::2, :].reshape(C, T // 2)
    ).astype(bf16)
    xp_odd = np.ascontiguousarray(
        xt_blocks[:, 1::2, :].reshape(C, T // 2)
    ).astype(bf16)

    # parity diag masks [KB, QB]: even keeps col>=p, odd keeps col>=128+p
    p = np.arange(KB)[:, None]
    col = np.arange(QB)[None, :]
    mask_even = (col >= p).astype(bf16)
    mask_odd = (col >= p + 128).astype(bf16)

    in_maps = []
    for c in range(N_CORES):
        hA = c
        hB = 8 + (c % 4)
        parity = 0 if c < 4 else 1

        def cols(w, h):
            return w[:, h * HD : (h + 1) * HD]

        wq = np.concatenate(
            [cols(w_attn[:, 0:C], hA), cols(w_attn[:, 0:C], hB)], axis=1
        ).astype(bf16)
        wk = np.concatenate(
            [cols(w_attn[:, C : 2 * C], hA), cols(w_attn[:, C : 2 * C], hB)], axis=1
        ).astype(bf16)
        wv = np.concatenate(
            [cols(w_attn[:, 2 * C : 3 * C], hA), cols(w_attn[:, 2 * C : 3 * C], hB)],
            axis=1,
        ).astype(bf16)
        wp = np.ascontiguousarray(w_proj[hA * HD : (hA + 1) * HD, :]).astype(bf16)
        in_maps.append(
            {
                "xt": xt,
                "xp": xp_even if parity == 0 else xp_odd,
                "wq": np.ascontiguousarray(wq),
                "wk": np.ascontiguousarray(wk),
                "wv": np.ascontiguousarray(wv),
                "wp": wp,
                "maskb": mask_even if parity == 0 else mask_odd,
            }
        )
    return in_maps


def _get_runner():
    """Build the shard_map'd PJRT executable once and reuse it across calls."""
    if "runner" in _CACHE:
        return _CACHE["runner"]
    import jax
    import concourse.mybir as mybir
    from concourse import bass2jax
    from jax.experimental.shard_map import shard_map
    from jax.sharding import Mesh, PartitionSpec

    nc = _get_nc()
    bass2jax.install_neuronx_cc_hook()

    in_names, out_names, out_avals, zero_outs = [], [], [], []
    for alloc in nc.m.functions[0].allocations:
        if not isinstance(alloc, mybir.MemoryLocationSet):
            continue
        name = alloc.memorylocations[0].name
        if alloc.kind == "ExternalInput":
            if nc.partition_id_tensor and name == nc.partition_id_tensor.name:
                continue
            in_names.append(name)
        elif alloc.kind == "ExternalOutput":
            shape = tuple(alloc.tensor_shape)
            dtype = mybir.dt.np(alloc.dtype)
            out_names.append(name)
            out_avals.append(jax.core.ShapedArray(shape, dtype))
            zero_outs.append(np.zeros(shape, dtype))
    n_params = len(in_names)
    all_in_names = in_names + out_names
    if nc.partition_id_tensor:
        all_in_names = all_in_names + [nc.partition_id_tensor.name]

    def _body(*args):
        operands = list(args)
        if nc.partition_id_tensor:
            operands.append(bass2jax.partition_id_tensor())
        outs = bass2jax._bass_exec_p.bind(
            *operands,
            out_avals=tuple(out_avals),
            in_names=tuple(all_in_names),
            out_names=tuple(out_names),
            lowering_input_output_aliases=(),
            sim_require_finite=True,
            sim_require_nnan=True,
            nc=nc,
        )
        return tuple(outs)

    devices = jax.devices()[:N_CORES]
    mesh = Mesh(np.asarray(devices), ("core",))
    n_out = len(out_names)
    donate = tuple(range(n_params, n_params + n_out))
    sharded = jax.jit(
        shard_map(
            _body,
            mesh=mesh,
            in_specs=(PartitionSpec("core"),) * (n_params + n_out),
            out_specs=(PartitionSpec("core"),) * n_out,
            check_rep=False,
        ),
        donate_argnums=donate,
        keep_unused=True,
    )

    def run(in_maps):
        concat_in = [
            np.concatenate([in_maps[c][name] for c in range(N_CORES)], axis=0)
            for name in in_names
        ]
        concat_zeros = [
            np.zeros((N_CORES * z.shape[0], *z.shape[1:]), z.dtype)
            for z in zero_outs
        ]
        out_arrs = sharded(*concat_in, *concat_zeros)
        return [
            {
                name: np.asarray(out_arrs[i]).reshape(
                    N_CORES, *out_avals[i].shape
                )[c]
                for i, name in enumerate(out_names)
            }
            for c in range(N_CORES)
        ]

    _CACHE["runner"] = run
    return run


def kernel(x, w_attn, w_proj):
    run = _get_runner()
    w_proj_f32 = np.asarray(w_proj, dtype=np.float32)
    in_maps = _core_inputs(np.asarray(x), np.asarray(w_attn), w_proj_f32)
    results = run(in_maps)

    out = np.zeros((T, C), dtype=np.float32)
    for c in range(N_CORES):
        out += results[c]["out"].astype(np.float32)

    # heads 8-11: combine parity partials, then project on host (fp32)
    Y = np.empty((T, 4 * HD), dtype=np.float32)
    for j in range(4):
        e = results[j]["yb"].astype(np.float32)
        o = results[4 + j]["yb"].astype(np.float32)
        num = e[0:HD, :] + o[0:HD, :]
        den = e[HD : HD + 1, :] + o[HD : HD + 1, :]
        Y[:, j * HD : (j + 1) * HD] = (num / den).T
    out += Y @ w_proj_f32[8 * HD : 12 * HD, :]
    return out.reshape(1, T, C)
